# revision 119
# baseline (speedup 1.0000x reference)
"""Trainium2 Bass kernel for nn_MESGM_15857019256842.

Data-parallel over batch: 16 batches -> 8 cores x 2 batches.
Per core: gather clause tokens (indirect DMA, bf16), 2 GCN layers with
pooling fused into the per-group loop, projection, 8-head self-attention
over 2x32 clauses, FFN, label decoder, soft-label KL loss.
Each core emits (sum kl*mask, sum mask); host combines.

Host-side prep packs all weights into bf16 DRAM arrays laid out exactly
as the SBUF tiles want them (few large DMAs), pre-builds the transposed
block-diagonal adjacency, and pre-casts the encoder output to bf16.
Attention/projection weights prefetch during the GCN phase; FFN weights
prefetch during attention.
"""
import sys
sys.path.insert(0, '/opt/trn_rl_repo')
import numpy as np

from concourse import bass, mybir, tile
from concourse import bass_utils
from concourse.masks import make_identity
from concourse.vector_clock import ScopedClock

F32 = mybir.dt.float32
BF16 = mybir.dt.bfloat16
I32 = mybir.dt.int32
AF = mybir.ActivationFunctionType
AX = mybir.AxisListType
ALU = mybir.AluOpType

B, S, H, M, LC, NL, II, NH, DH = 16, 512, 768, 32, 32, 7, 3072, 8, 96
NCORES = 8
BB = B // NCORES          # 2 batches per core
NCL = BB * M              # 64 clauses per core
NROW = NCL * LC           # 2048 clause-token rows per core
RT = NROW // 128          # 16 row tiles
HC = H // 128             # 6 H chunks
IC = II // 128            # 24 intermediate chunks
LN_EPS = 1e-12
SQD = float(np.sqrt(DH))

# column layout of the packed small-constants tile [128, 100] f32
SP_GB1, SP_GB2, SP_PJB, SP_QB, SP_KB, SP_INTB, SP_DW = 0, 6, 12, 18, 26, 34, 58
# column layout of the broadcast-bias pack [5383] f32
BB_AOB, BB_VB, BB_OUTB, BB_L1G, BB_L1B, BB_L2G, BB_L2B, BB_DECB = (
    0, 768, 1536, 2304, 3072, 3840, 4608, 5376)
NBB = 5383

_MAX_WAITS = 1


def _patched_drain_and_barrier(self, tick_clock, wait_clock):
    nc = self.nc
    drain_inst = nc.sync.drain()
    wait_clock.add_sem_waits(
        drain_inst.ins, ScopedClock({None: tick_clock.global_clock})
    )
    si = drain_inst.ins.sync_info
    waits = list(si.on_wait or [])
    if len(waits) > _MAX_WAITS:
        si.on_wait = waits[:_MAX_WAITS]
        rest = waits[_MAX_WAITS:]
        for i in range(0, len(rest), _MAX_WAITS):
            nop = nc.sync.nop(nofuse=True)
            nop.ins.sync_info = mybir.SyncInfo(
                on_wait=rest[i : i + _MAX_WAITS], on_update=[]
            )
    nc.all_engine_barrier()
    assert self.sems is not None
    popped = nc._tile_sem_poison_stack.pop()
    assert popped is self._sem_poison
    nc.clear_and_free_semaphores(list(self.sems.allocated().values()))
    nc.all_engine_barrier()


tile.TileContext._drain_and_barrier = _patched_drain_and_barrier


def legalize_waits(nc, limit=1):
    """TRN2 instructions carry at most one sem wait; hoist extras onto nops."""
    nfix = 0
    for blk in nc.main_func.blocks:
        insts = list(blk.instructions)
        pos = 0
        for inst in insts:
            si = inst.sync_info
            waits = list(si.on_wait) if si is not None and si.on_wait else []
            if len(waits) > limit:
                si.on_wait = waits[-limit:]
                rest = waits[:-limit]
                eng = nc.engines[inst.engine]
                for j in range(0, len(rest), limit):
                    nop = eng.nop(nofuse=True)
                    nop.ins.sync_info = mybir.SyncInfo(
                        on_wait=rest[j : j + limit], on_update=[]
                    )
                    src_blk = nc.cur_bb.bb
                    popped = src_blk.instructions.pop()
                    assert popped.name == nop.ins.name
                    blk.instructions.insert(pos, nop.ins)
                    pos += 1
                nfix += 1
            pos += 1
    return nfix


def build_program():
    nc = bass.Bass(trn_type="TRN2")

    # ---- DRAM I/O (everything pre-packed on host) ------------------------
    # xtg holds the pre-gathered masked tokens already transposed into
    # [128 h-part, group, h-chunk, 512 rows] tile layout
    xtg = nc.dram_tensor("xtg", [128, RT * H], BF16, kind="ExternalInput")
    wrmb = nc.dram_tensor("wrmb", [NROW], BF16, kind="ExternalInput")
    aux = nc.dram_tensor("aux", [128, NCL + HC * NL], BF16,
                         kind="ExternalInput")
    adjt = nc.dram_tensor("adjt", [128, RT * 128], BF16, kind="ExternalInput")
    wg = nc.dram_tensor("wg", [128, 2 * HC * H], BF16, kind="ExternalInput")
    wa = nc.dram_tensor("wa", [128, (24 + 4 * HC) * H], BF16, kind="ExternalInput")
    wb = nc.dram_tensor("wb", [128, HC * II + IC * H], BF16, kind="ExternalInput")
    smallpk = nc.dram_tensor("smallpk", [128, 100], F32, kind="ExternalInput")
    biasbc = nc.dram_tensor("biasbc", [NBB], F32, kind="ExternalInput")
    percl = nc.dram_tensor("percl", [NCL, 520], F32, kind="ExternalInput")
    out_d = nc.dram_tensor("out", [NCL], F32, kind="ExternalOutput")

    with tile.TileContext(nc) as tc:
        _body(nc, tc, xtg, wrmb, aux, adjt, wg, wa, wb, smallpk,
              biasbc, percl, out_d)

    nfix = legalize_waits(nc)
    return nc, nfix


def _body(nc, tc, xtg, wrmb, aux, adjt, wg, wa, wb, smallpk,
          biasbc, percl, out_d):
    from contextlib import ExitStack
    ctx = ExitStack()
    with ctx:
        pp = ctx.enter_context(tc.tile_pool(name="persist", bufs=1))

        ident = pp.tile([128, 128], F32, tag="ident")
        make_identity(nc, ident[:])
        ident_b = pp.tile([128, 128], BF16, tag="identb")
        nc.vector.tensor_copy(out=ident_b[:], in_=ident[:])

        sp = pp.tile([128, 100], F32, tag="smallpk")
        nc.sync.dma_start(out=sp[:], in_=smallpk[:, :])

        # warm the natural_log_exp table set (covers every transcendental
        # we use except gelu) while the first DMAs stream
        warm = pp.tile([128, 1], F32, tag="actwarm")
        nc.vector.memset(warm[:], 0.5)
        nc.scalar.activation(out=warm[:], in_=warm[:], func=AF.Ln)
        nc.scalar.activation(out=warm[:], in_=warm[:], func=AF.Exp)
        nc.scalar.mul(out=sp[:DH, SP_QB : SP_QB + NH],
                      in_=sp[:DH, SP_QB : SP_QB + NH], mul=1.0 / SQD)

        PT = pp.tile([128, 24, NCL], BF16, tag="PT")
        eps_t = pp.tile([NCL, 1], F32, tag="epst")
        nc.vector.memset(eps_t[:], LN_EPS)


        bb_t = pp.tile([NCL, NBB], F32, tag="biasbc")
        pcl = pp.tile([NCL, 520], F32, tag="percl")
        amask8 = pcl[:, 0:512].rearrange("p (h n) -> p h n", h=NH)
        tgt_sb = pcl[:, 512:519]
        cnm_pp = pcl[:, 519:520]
        # t*ln(t) (and its row-sum) for the KL tail, computed in phase 1
        a1_early = pp.tile([NCL, NL], F32, tag="a1early")
        s_a1e = pp.tile([NCL, 1], F32, tag="sa1e")
        # replicated 1/len per clause + bf16 decoder weights (tiny)
        aux_t = pp.tile([128, NCL + HC * NL], BF16, tag="aux")
        lens_r = aux_t[:, 0:NCL]

        # attention/projection weights tile (resident through phase 2).
        # DMAs are issued on the scalar HWDGE queue after GCN group 0 so
        # they don't compete with the critical-path loads early on.
        WAC = (24 + 4 * HC) * H
        wa_t = ctx.enter_context(tc.tile_pool(name="wa", bufs=1)).tile(
            [128, WAC], BF16, tag="wa")

        def issue_wa_loads():
            # delayed behind a virtual timestamp so these (large, non-
            # critical) loads don't steal DMA bandwidth from the phase-1
            # critical path (the scheduler otherwise hoists dep-free DMAs
            # to the very front)
            with tc.tile_wait_until(0.015):
                nc.sync.dma_start(out=bb_t[:],
                                  in_=bass.AP(tensor=biasbc, offset=0,
                                              ap=[[0, NCL], [1, NBB]]))
                nc.sync.dma_start(out=pcl[:], in_=percl[:, :])
                for j in range(6):
                    c0, c1 = j * (WAC // 6), (j + 1) * (WAC // 6)
                    nc.sync.dma_start(out=wa_t[:, c0:c1],
                                      in_=bass.AP(tensor=wa, offset=c0,
                                                  ap=[[WAC, 128], [1, c1 - c0]]))
                # KL's t*ln(t) term is independent of everything else
                lnt = pscr.tile([NCL, NL], F32, tag="lnt")
                nc.scalar.activation(out=lnt[:], in_=tgt_sb, func=AF.Ln)
                nc.vector.tensor_tensor(out=a1_early[:], in0=tgt_sb, in1=lnt[:],
                                        op=ALU.mult)
                nc.vector.reduce_sum(out=s_a1e[:], in_=a1_early[:], axis=AX.X)

        projw = wa_t[:, 0 : 24 * H]
        qw = wa_t[:, 24 * H : 30 * H]
        kw = wa_t[:, 30 * H : 36 * H]
        vw = wa_t[:, 36 * H : 42 * H]
        aow = wa_t[:, 42 * H : 48 * H]

        # proj psum lives across layer 2 so its X-column accumulation
        # steps can interleave with the GCN as PE gap-filler
        pjps = ctx.enter_context(tc.tile_pool(name="pjps", bufs=1, space="PSUM"))
        pcs = pjps.tile([128, HC, NCL], F32, tag="pj")
        korder = (list(range(0, 6)) + list(range(12, 18))
                  + list(range(6, 12)) + list(range(18, 24)))

        def proj_steps(kis):
            for ki in kis:
                k = korder[ki]
                for m in range(HC):
                    nc.tensor.matmul(
                        out=pcs[:, m, :],
                        lhsT=projw[:, k * H + m * 128 : k * H + m * 128 + 128],
                        rhs=PT[:, k, :], start=(ki == 0), stop=(ki == 23))

        # =================== phase 1: gather + GCN + pooling ==============
        ph1 = ExitStack()
        p1p = ph1.enter_context(tc.tile_pool(name="p1misc", bufs=1))
        xmt = ph1.enter_context(tc.tile_pool(name="xmt", bufs=3))
        wg_t = p1p.tile([128, 2 * HC * H], BF16, tag="wg")
        adjT = p1p.tile([128, RT, 128], BF16, tag="adjT")
        wrm_bcb = p1p.tile([128, NROW], BF16, tag="wrmbcb")

        # hand-ordered sync-queue loads: token group 0 first, then gc1,
        # remaining token groups interleaved with the rest.
        xmts = []
        for g in range(4):
            xmts.append(xmt.tile([128, HC, 512], BF16, tag="xmt", name=f"xmt{g}"))

        def xmt_load(g):
            nc.sync.dma_start(out=xmts[g][:],
                              in_=bass.AP(tensor=xtg, offset=g * HC * 512,
                                          ap=[[RT * H, 128], [1, HC * 512]]))

        xmt_load(0)
        nc.sync.dma_start(out=wg_t[:, 0 : HC * H],
                          in_=bass.AP(tensor=wg, offset=0,
                                      ap=[[2 * HC * H, 128], [1, HC * H]]))
        xmt_load(1)
        xmt_load(2)
        nc.sync.dma_start(out=adjT[:], in_=adjt[:, :])
        nc.sync.dma_start(out=wg_t[:, HC * H : 2 * HC * H],
                          in_=bass.AP(tensor=wg, offset=HC * H,
                                      ap=[[2 * HC * H, 128], [1, HC * H]]))
        nc.sync.dma_start(out=wrm_bcb[:],
                          in_=bass.AP(tensor=wrmb, offset=0,
                                      ap=[[0, 128], [1, NROW]]))
        nc.sync.dma_start(out=aux_t[:], in_=aux[:, :])
        xmt_load(3)

        big = ph1.enter_context(tc.tile_pool(name="big", bufs=1))
        H1T = big.tile([128, HC, NROW], BF16, tag="H1T")
        ynp = ph1.enter_context(tc.tile_pool(name="ynp", bufs=2))
        h2p = ph1.enter_context(tc.tile_pool(name="h2p", bufs=2))
        pscr = ph1.enter_context(tc.tile_pool(name="pscr", bufs=4))
        gps1 = ph1.enter_context(tc.tile_pool(name="gps1", bufs=3, space="PSUM"))
        gps2 = ph1.enter_context(tc.tile_pool(name="gps2", bufs=2, space="PSUM"))
        zps = ph1.enter_context(tc.tile_pool(name="zps", bufs=2, space="PSUM"))

        def y_block(XT, xoff, wofs, g, tag):
            """XT[:, c, xoff:xoff+512] @ W -> 4 row tiles of y, bf16 SBUF."""
            yns = []
            for rr in range(4):
                p1 = gps1.tile([128, 512], F32, tag="y1", name=f"y1_{tag}{g}_{rr}")
                p2 = gps2.tile([128, 256], F32, tag="y2", name=f"y2_{tag}{g}_{rr}")
                for c in range(HC):
                    lhs = XT[:, c, xoff + rr * 128 : xoff + rr * 128 + 128]
                    nc.tensor.matmul(out=p1[:], lhsT=lhs,
                                     rhs=wg_t[:, wofs + c * H : wofs + c * H + 512],
                                     start=(c == 0), stop=(c == HC - 1))
                    nc.tensor.matmul(out=p2[:], lhsT=lhs,
                                     rhs=wg_t[:, wofs + c * H + 512 : wofs + c * H + 768],
                                     start=(c == 0), stop=(c == HC - 1))
                yr = ynp.tile([128, H], BF16, tag=f"yn{rr}", name=f"yn_{tag}{g}_{rr}")
                nc.scalar.copy(out=yr[:, 0:512], in_=p1[:])
                nc.vector.tensor_copy(out=yr[:, 512:768], in_=p2[:])
                yns.append(yr)
            return yns

        def z_block(yns, g, bcol, HT, hoff, tag):
            """adj @ y -> relu(. + b) into HT[:, c, hoff:hoff+512] (bf16)."""
            for c in range(HC):
                zp = zps.tile([128, 512], F32, tag="z", name=f"z_{tag}{g}_{c}")
                for rr in range(4):
                    nc.tensor.matmul(
                        out=zp[:, rr * 128 : rr * 128 + 128],
                        lhsT=yns[rr][:, c * 128 : c * 128 + 128],
                        rhs=adjT[:, 4 * g + rr, :],
                        start=True, stop=True,
                    )
                nc.scalar.activation(
                    out=HT[:, c, hoff : hoff + 512], in_=zp[:],
                    func=AF.Relu, bias=sp[:, bcol + c : bcol + c + 1], scale=1.0,
                )

        def pool_group(XT, xoff, g, mcol, acol):
            """max/avg pool 512 cols (16 clauses) of XT into PT."""
            for c in range(HC):
                v = XT[:, c, xoff : xoff + 512].rearrange("p (n l) -> p n l", l=LC)
                nc.vector.reduce_max(out=PT[:, mcol + c, 16 * g : 16 * g + 16],
                                     in_=v, axis=AX.X)
                st = pscr.tile([128, 16], F32, tag="pst", name=f"pst{mcol}_{g}_{c}")
                nc.vector.reduce_sum(out=st[:], in_=v, axis=AX.X)
                nc.gpsimd.tensor_tensor(
                    out=PT[:, acol + c, 16 * g : 16 * g + 16], in0=st[:],
                    in1=lens_r[:, 16 * g : 16 * g + 16], op=ALU.mult)

        # ---- layer 1 -----------------------------------------------------
        for g in range(4):
            if g == 1:
                issue_wa_loads()
            XmT = xmts[g]
            pool_group(XmT, 0, g, 0, 12)
            yns = y_block(XmT, 0, 0, g, "l1")
            z_block(yns, g, SP_GB1, H1T, g * 512, "l1")

        # ---- layer 2 (H2 pooled on the fly, never materialized) ----------
        # proj accumulation over the X-pool columns, emitted before ANY
        # l2 pool write so the access history carries no false deps; the
        # PE works through these during l2's vector-bound stretches
        proj_steps(range(12))
        for g in range(4):
            yns = y_block(H1T, g * 512, HC * H, g, "l2")
            h2g = h2p.tile([128, HC, 512], BF16, tag="h2g", name=f"h2g{g}")
            z_block(yns, g, SP_GB2, h2g, 0, "l2")
            for c in range(HC):
                nc.gpsimd.tensor_tensor(out=h2g[:, c, :], in0=h2g[:, c, :],
                                        in1=wrm_bcb[:, g * 512 : g * 512 + 512],
                                        op=ALU.mult)
            pool_group(h2g, 0, g, 6, 18)

        ph1.close()

        # =================== phase 2: proj + attention + FFN ==============
        ph2 = ExitStack()
        wbp = ph2.enter_context(tc.tile_pool(name="wbp", bufs=1))
        WBC = HC * II + IC * H
        wb_t = wbp.tile([128, WBC], BF16, tag="wb")
        for j in range(6):
            c0, c1 = j * (WBC // 6), (j + 1) * (WBC // 6)
            nc.sync.dma_start(out=wb_t[:, c0:c1],
                              in_=bass.AP(tensor=wb, offset=c0,
                                          ap=[[WBC, 128], [1, c1 - c0]]))
        intw = wb_t[:, 0 : HC * II]
        outw = wb_t[:, HC * II :]

        at = ph2.enter_context(tc.tile_pool(name="attn", bufs=1))
        sc = ph2.enter_context(tc.tile_pool(name="scr", bufs=1))

        # projection: finish the accumulation (H2-pool columns), then relu
        cvT = at.tile([128, HC, NCL], BF16, tag="cvT")
        proj_steps(range(12, 24))
        for m in range(HC):
            nc.scalar.activation(out=cvT[:, m, :], in_=pcs[:, m, :], func=AF.Relu,
                                 bias=sp[:, SP_PJB + m : SP_PJB + m + 1], scale=1.0)

        # cv natural + ao_b (residual base)
        cv_pa = at.tile([NCL, H], F32, tag="cvpa")
        with tc.tile_pool(name="cvt2", bufs=3, space="PSUM") as cvt2:
            for c in range(HC):
                ps = cvt2.tile([NCL, 128], BF16, tag="cvn", name=f"cvn{c}")
                nc.tensor.transpose(out=ps[:], in_=cvT[:, c, :], identity=ident_b[:])
                nc.vector.tensor_tensor(out=cv_pa[:, c * 128 : c * 128 + 128], in0=ps[:],
                                        in1=bb_t[:, BB_AOB + c * 128 : BB_AOB + c * 128 + 128],
                                        op=ALU.add)

        # attention
        QT = at.tile([DH, NH, NCL], BF16, tag="QT")
        KT = at.tile([DH, NH, NCL], BF16, tag="KT")
        Vn = at.tile([NCL, H], BF16, tag="Vn")
        ctx_nat = at.tile([NCL, H], BF16, tag="ctxn")
        att8 = at.tile([NCL, NH, NCL], BF16, tag="att8")
        sums_t = at.tile([NCL, NH], F32, tag="sums")
        recip_t = at.tile([NCL, NH], F32, tag="recip")
        s2 = at.tile([NCL, NH, NCL], F32, tag="s2")
        negmax = at.tile([NCL, NH], F32, tag="negmax")

        with tc.tile_pool(name="qkps", bufs=2, space="PSUM") as qkps, \
             tc.tile_pool(name="vps", bufs=1, space="PSUM") as vps, \
             tc.tile_pool(name="scps", bufs=1, space="PSUM") as scps:
            psq = qkps.tile([DH, NH * NCL], F32, tag="qk", name="psq")
            for h in range(NH):
                for c in range(HC):
                    nc.tensor.matmul(out=psq[:, h * NCL : h * NCL + NCL],
                                     lhsT=qw[:, c * H + h * DH : c * H + h * DH + DH],
                                     rhs=cvT[:, c, :], start=(c == 0), stop=(c == HC - 1))
            for h in range(NH):
                nc.scalar.activation(out=QT[:, h, :], in_=psq[:, h * NCL : h * NCL + NCL],
                                     func=AF.Identity,
                                     bias=sp[:DH, SP_QB + h : SP_QB + h + 1], scale=1.0 / SQD)
            psk = qkps.tile([DH, NH * NCL], F32, tag="qk", name="psk")
            for h in range(NH):
                for c in range(HC):
                    nc.tensor.matmul(out=psk[:, h * NCL : h * NCL + NCL],
                                     lhsT=kw[:, c * H + h * DH : c * H + h * DH + DH],
                                     rhs=cvT[:, c, :], start=(c == 0), stop=(c == HC - 1))
            for h in range(NH):
                nc.scalar.activation(out=KT[:, h, :], in_=psk[:, h * NCL : h * NCL + NCL],
                                     func=AF.Identity,
                                     bias=sp[:DH, SP_KB + h : SP_KB + h + 1], scale=1.0)

            pv1 = vps.tile([NCL, 512], F32, tag="v1")
            pv2 = vps.tile([NCL, 256], F32, tag="v2")
            for c in range(HC):
                nc.tensor.matmul(out=pv1[:], lhsT=cvT[:, c, :],
                                 rhs=vw[:, c * H : c * H + 512],
                                 start=(c == 0), stop=(c == HC - 1))
                nc.tensor.matmul(out=pv2[:], lhsT=cvT[:, c, :],
                                 rhs=vw[:, c * H + 512 : c * H + 768],
                                 start=(c == 0), stop=(c == HC - 1))
            nc.vector.tensor_tensor(out=Vn[:, 0:512], in0=pv1[:],
                                    in1=bb_t[:, BB_VB : BB_VB + 512], op=ALU.add)
            nc.vector.tensor_tensor(out=Vn[:, 512:768], in0=pv2[:],
                                    in1=bb_t[:, BB_VB + 512 : BB_VB + 768], op=ALU.add)

            pss = scps.tile([NCL, NH * NCL], F32, tag="scores")
            for h in range(NH):
                nc.tensor.matmul(out=pss[:, h * NCL : h * NCL + NCL], lhsT=QT[:, h, :],
                                 rhs=KT[:, h, :], start=True, stop=True)
            nc.vector.tensor_tensor(out=s2[:], in0=pss[:].rearrange("p (h n) -> p h n", h=NH),
                                    in1=amask8, op=ALU.add)
        nc.vector.tensor_reduce(out=negmax[:], in_=s2[:], axis=AX.X, op=ALU.max,
                                negate=True)
        for h in range(NH):
            nc.scalar.activation(
                out=att8[:, h, :], in_=s2[:, h, :], func=AF.Exp,
                bias=negmax[:, h : h + 1], scale=1.0,
                accum_out=sums_t[:, h : h + 1],
            )
        nc.vector.reciprocal(out=recip_t[:], in_=sums_t[:])
        with tc.tile_pool(name="ctps", bufs=3, space="PSUM") as ctps, \
             tc.tile_pool(name="atts", bufs=3) as atts:
            for h in range(NH):
                pst = ctps.tile([NCL, NCL], BF16, tag="attT", name=f"attT{h}")
                nc.tensor.transpose(out=pst[:], in_=att8[:, h, :], identity=ident_b[:64, :64])
                asb = atts.tile([NCL, NCL], BF16, tag="attTs", name=f"attTs{h}")
                nc.vector.tensor_copy(out=asb[:], in_=pst[:])
                pctx = ctps.tile([NCL, DH], F32, tag="ctx", name=f"ctx{h}")
                nc.tensor.matmul(out=pctx[:], lhsT=asb[:], rhs=Vn[:, h * DH : h * DH + DH],
                                 start=True, stop=True)
                nc.scalar.mul(out=ctx_nat[:, h * DH : h * DH + DH], in_=pctx[:],
                              mul=recip_t[:, h : h + 1])

        def transpose_to(src_nat, dstT, dt=F32):
            idn = ident if dt == F32 else ident_b
            with tc.tile_pool(name="trp", bufs=3, space="PSUM") as trp:
                for c in range(HC):
                    ps = trp.tile([128, NCL], dt, tag="trt", name=f"trt{c}")
                    nc.tensor.transpose(out=ps[:], in_=src_nat[:, c * 128 : c * 128 + 128],
                                        identity=idn[:64, :64])
                    if c % 2 == 0:
                        nc.vector.tensor_copy(out=dstT[:, c, :], in_=ps[:])
                    else:
                        nc.scalar.copy(out=dstT[:, c, :], in_=ps[:])

        def layer_norm(x_nat, gcol, bcol, y_nat):
            with tc.tile_pool(name="lnp", bufs=1) as lnp:
                stats = lnp.tile([NCL, 3, 6], F32, tag="lnstats")
                for i in range(3):
                    nc.vector.bn_stats(out=stats[:, i, :], in_=x_nat[:, i * 256 : i * 256 + 256])
                mv = lnp.tile([NCL, 2], F32, tag="lnmv")
                nc.vector.bn_aggr(out=mv[:], in_=stats[:])
                # rstd = exp(-0.5*ln(var+eps)) — stays in the natlog_exp
                # table set (a Sqrt would force a table switch)
                lv = lnp.tile([NCL, 1], F32, tag="lnlv")
                nc.scalar.activation(out=lv[:], in_=mv[:, 1:2], func=AF.Ln, bias=eps_t[:, :1], scale=1.0)
                rstd = lnp.tile([NCL, 1], F32, tag="lnrstd")
                nc.scalar.activation(out=rstd[:], in_=lv[:], func=AF.Exp, scale=-0.5)
                # normalize in place (x_nat is dead after), elementwise
                # tail split across DVE and GpSimd
                nc.vector.tensor_scalar(out=x_nat[:, 0:512], in0=x_nat[:, 0:512],
                                        scalar1=mv[:, 0:1], scalar2=rstd[:, :1],
                                        op0=ALU.subtract, op1=ALU.mult)
                nc.gpsimd.tensor_scalar(out=x_nat[:, 512:768], in0=x_nat[:, 512:768],
                                        scalar1=mv[:, 0:1], scalar2=rstd[:, :1],
                                        op0=ALU.subtract, op1=ALU.mult)
                nc.vector.tensor_tensor(out=x_nat[:, 0:512], in0=x_nat[:, 0:512],
                                        in1=bb_t[:, gcol : gcol + 512], op=ALU.mult)
                nc.gpsimd.tensor_tensor(out=x_nat[:, 512:768], in0=x_nat[:, 512:768],
                                        in1=bb_t[:, gcol + 512 : gcol + H], op=ALU.mult)
                nc.vector.tensor_tensor(out=y_nat[:, 0:512], in0=x_nat[:, 0:512],
                                        in1=bb_t[:, bcol : bcol + 512], op=ALU.add)
                nc.gpsimd.tensor_tensor(out=y_nat[:, 512:768], in0=x_nat[:, 512:768],
                                        in1=bb_t[:, bcol + 512 : bcol + H], op=ALU.add)

        # ao proj + residual + LN1
        ctxT = at.tile([128, HC, NCL], BF16, tag="ctxT")
        transpose_to(ctx_nat, ctxT, dt=BF16)
        attn_out = at.tile([NCL, H], F32, tag="attnout")
        with tc.tile_pool(name="aops", bufs=1, space="PSUM") as aops:
            pa1 = aops.tile([NCL, 512], F32, tag="ao1")
            pa2 = aops.tile([NCL, 256], F32, tag="ao2")
            for c in range(HC):
                nc.tensor.matmul(out=pa1[:], lhsT=ctxT[:, c, :],
                                 rhs=aow[:, c * H : c * H + 512],
                                 start=(c == 0), stop=(c == HC - 1))
                nc.tensor.matmul(out=pa2[:], lhsT=ctxT[:, c, :],
                                 rhs=aow[:, c * H + 512 : c * H + 768],
                                 start=(c == 0), stop=(c == HC - 1))
            ln_in = sc.tile([NCL, H], F32, tag="lnin1")
            nc.vector.tensor_tensor(out=ln_in[:, 0:512], in0=pa1[:], in1=cv_pa[:, 0:512], op=ALU.add)
            nc.vector.tensor_tensor(out=ln_in[:, 512:768], in0=pa2[:], in1=cv_pa[:, 512:768], op=ALU.add)
            layer_norm(ln_in, BB_L1G, BB_L1B, attn_out)
        # dummy gelu: pulls the gelu table load off the critical path,
        # overlapping it with the aoT transposes + FFN1 matmuls
        nc.scalar.activation(out=warm[:], in_=warm[:], func=AF.Gelu)

        # FFN1 computed transposed: interT[:, j, :] = gelu(int_w[:, j]^T @ ao + b_j)
        aoT = at.tile([128, HC, NCL], BF16, tag="aoT")
        transpose_to(attn_out, aoT)
        interT = at.tile([128, IC, NCL], BF16, tag="interT")
        out_nat = at.tile([NCL, H], BF16, tag="outnat")
        with tc.tile_pool(name="fps", bufs=1, space="PSUM") as fps, \
             tc.tile_pool(name="ops", bufs=1, space="PSUM") as ops:
            psf = [fps.tile([128, 8, NCL], F32, tag=f"fi{n}", name=f"fi{n}") for n in range(3)]
            for j in range(IC):
                for c in range(HC):
                    nc.tensor.matmul(out=psf[j // 8][:, j % 8, :],
                                     lhsT=intw[:, c * II + j * 128 : c * II + j * 128 + 128],
                                     rhs=aoT[:, c, :], start=(c == 0), stop=(c == HC - 1))
            for j in range(IC):
                nc.scalar.activation(out=interT[:, j, :], in_=psf[j // 8][:, j % 8, :],
                                     func=AF.Gelu,
                                     bias=sp[:, SP_INTB + j : SP_INTB + j + 1], scale=1.0)
            # dummy ln: reloads natlog_exp during the FFN2 matmuls so
            # LN2/KL don't pay the table switch
            nc.scalar.activation(out=warm[:], in_=warm[:], func=AF.Ln)

            po1 = ops.tile([NCL, 512], F32, tag="o1")
            po2 = ops.tile([NCL, 256], F32, tag="o2")
            for cc in range(IC):
                nc.tensor.matmul(out=po1[:], lhsT=interT[:, cc, :],
                                 rhs=outw[:, cc * H : cc * H + 512],
                                 start=(cc == 0), stop=(cc == IC - 1))
                nc.tensor.matmul(out=po2[:], lhsT=interT[:, cc, :],
                                 rhs=outw[:, cc * H + 512 : cc * H + 768],
                                 start=(cc == 0), stop=(cc == IC - 1))
            ln_in2 = sc.tile([NCL, H], F32, tag="lnin2")
            nc.vector.tensor_tensor(out=ln_in2[:, 0:512], in0=po1[:], in1=attn_out[:, 0:512], op=ALU.add)
            nc.vector.tensor_tensor(out=ln_in2[:, 512:768], in0=po2[:], in1=attn_out[:, 512:768], op=ALU.add)
            nc.vector.tensor_tensor(out=ln_in2[:], in0=ln_in2[:],
                                    in1=bb_t[:, BB_OUTB : BB_OUTB + H], op=ALU.add)
            layer_norm(ln_in2, BB_L2G, BB_L2B, out_nat)

        # decoder + KL
        outT = at.tile([128, HC, NCL], BF16, tag="outT")
        transpose_to(out_nat, outT, dt=BF16)
        dwb = aux_t[:, NCL : NCL + HC * NL].rearrange("p (c l) -> p c l", l=NL)

        with tc.tile_pool(name="klps", bufs=1, space="PSUM") as klps, \
             tc.tile_pool(name="klsc", bufs=1) as klsc:
            pd = klps.tile([NCL, NL], F32, tag="pred")
            for c in range(HC):
                nc.tensor.matmul(out=pd[:], lhsT=outT[:, c, :], rhs=dwb[:, c, :],
                                 start=(c == 0), stop=(c == HC - 1))
            pred = klsc.tile([NCL, NL], F32, tag="pred_sb")
            nc.vector.tensor_tensor(out=pred[:], in0=pd[:],
                                    in1=bb_t[:, BB_DECB : BB_DECB + NL], op=ALU.add)
            negm = klsc.tile([NCL, 1], F32, tag="negm")
            nc.vector.tensor_reduce(out=negm[:], in_=pred[:], axis=AX.X, op=ALU.max, negate=True)
            # kl = sum(t*ln t) - sum(t*pred) - (lnS - negm)   [sum(t) == 1]
            esc = klsc.tile([NCL, NL], F32, tag="esc")
            ssum = klsc.tile([NCL, 1], F32, tag="ssum")
            nc.scalar.activation(out=esc[:], in_=pred[:], func=AF.Exp,
                                 bias=negm[:, :1], scale=1.0, accum_out=ssum[:, :1])
            lnS = klsc.tile([NCL, 1], F32, tag="lnS")
            nc.scalar.activation(out=lnS[:], in_=ssum[:], func=AF.Ln)
            c1 = klsc.tile([NCL, 1], F32, tag="c1")
            nc.scalar.activation(out=c1[:], in_=lnS[:], func=AF.Identity,
                                 bias=negm[:, :1], scale=-1.0)
            tp = klsc.tile([NCL, NL], F32, tag="tp")
            nc.vector.tensor_tensor(out=tp[:], in0=tgt_sb, in1=pred[:], op=ALU.mult)
            stp = klsc.tile([NCL, 1], F32, tag="stp")
            nc.vector.reduce_sum(out=stp[:], in_=tp[:], axis=AX.X)
            kd = klsc.tile([NCL, 1], F32, tag="kd")
            nc.vector.tensor_tensor(out=kd[:], in0=s_a1e[:], in1=stp[:], op=ALU.subtract)
            kl = klsc.tile([NCL, 1], F32, tag="kl")
            nc.vector.tensor_tensor(out=kl[:], in0=kd[:], in1=c1[:], op=ALU.subtract)
            # per-clause KL shipped raw; the host applies the clause mask
            # and does the final weighted mean
            nc.sync.dma_start(out=out_d[:, None], in_=kl[:])
        ph2.close()


_CACHE = {}


def _get_program():
    if "nc" not in _CACHE:
        nc, nfix = build_program()
        _CACHE["nc"] = nc
    return _CACHE["nc"]


def _chunk_pack(w_, nchunk):
    """[nchunk*128, cols] -> [128, nchunk*cols] with chunk k at cols k*cols."""
    cols = w_.shape[1]
    return np.ascontiguousarray(
        w_.reshape(nchunk, 128, cols).transpose(1, 0, 2).reshape(128, nchunk * cols))


def shard_inputs(inputs):
    import ml_dtypes
    BF = ml_dtypes.bfloat16
    enc = np.asarray(inputs["encoder_hs"], dtype=np.float32)
    wr = np.asarray(inputs["word_recovery"], dtype=np.int32)
    wm = np.asarray(inputs["word_recovery_mask"], dtype=np.int32)
    cn = np.asarray(inputs["clause_num_mask"], dtype=np.int32)
    adj = np.asarray(inputs["adj_matrix"], dtype=np.float32)
    tl = np.asarray(inputs["target_labels"], dtype=np.float32)

    f32 = lambda k: np.asarray(inputs[k], dtype=np.float32)

    # ---- shared weight packs (identical across cores) --------------------
    wg_pk = np.concatenate([
        _chunk_pack(f32("gc1_w"), HC), _chunk_pack(f32("gc2_w"), HC)], axis=1).astype(BF)
    wa_pk = np.concatenate([
        _chunk_pack(f32("proj_w"), 24),
        _chunk_pack(f32("q_w"), HC), _chunk_pack(f32("k_w"), HC),
        _chunk_pack(f32("v_w"), HC), _chunk_pack(f32("ao_w"), HC)], axis=1).astype(BF)
    wb_pk = np.concatenate([
        _chunk_pack(f32("int_w"), HC), _chunk_pack(f32("out_w"), IC)], axis=1).astype(BF)

    smallpk = np.zeros((128, 100), dtype=np.float32)
    smallpk[:, SP_GB1:SP_GB1 + 6] = f32("gc1_b").reshape(6, 128).T
    smallpk[:, SP_GB2:SP_GB2 + 6] = f32("gc2_b").reshape(6, 128).T
    smallpk[:, SP_PJB:SP_PJB + 6] = f32("proj_b").reshape(6, 128).T
    smallpk[:DH, SP_QB:SP_QB + NH] = f32("q_b").reshape(NH, DH).T
    smallpk[:DH, SP_KB:SP_KB + NH] = f32("k_b").reshape(NH, DH).T
    smallpk[:, SP_INTB:SP_INTB + IC] = f32("int_b").reshape(IC, 128).T
    smallpk[:, SP_DW:SP_DW + HC * NL] = f32("dec_w").reshape(HC, 128, NL).transpose(1, 0, 2).reshape(128, HC * NL)

    biasbc = np.concatenate([
        f32("ao_b"), f32("v_b"), f32("out_b"),
        f32("ln1_g"), f32("ln1_b"), f32("ln2_g"), f32("ln2_b"),
        f32("dec_b")]).astype(np.float32)
    assert biasbc.shape[0] == NBB

    in_maps = []
    boff = (np.arange(BB) * S).astype(np.int32)[:, None, None]
    for i in range(NCORES):
        sl = slice(BB * i, BB * i + BB)
        cnm_i = cn[sl].astype(np.float32).reshape(NCL)
        # attention mask: -1e4 for masked clauses within a batch block,
        # -3e4 for cross-batch entries (forces exp() to exactly 0).
        am = np.full((NCL, NCL), -30000.0, dtype=np.float32)
        for b in range(BB):
            blk = (1.0 - cnm_i[b * M : (b + 1) * M]) * -10000.0
            am[b * M : (b + 1) * M, b * M : (b + 1) * M] = blk[None, :]
        amask8 = np.repeat(am[:, None, :], NH, axis=1).reshape(NCL, NH * NCL)
        percl = np.concatenate([
            amask8, tl[sl].reshape(NCL, NL), cnm_i[:, None]], axis=1)

        gidx = (wr[sl] + boff).reshape(NROW).astype(np.int32)
        wrm_f = wm[sl].astype(np.float32).reshape(NROW)

        # pre-gathered, masked clause tokens, already transposed to the
        # XmT layout: [128 h-part, group, h-chunk, 512 rows]
        xtg_full = enc[sl].reshape(BB * S, H)[gidx] * wrm_f[:, None]
        xtg_h = xtg_full.reshape(4, 512, HC, 128).transpose(3, 0, 2, 1).reshape(
            128, RT * H).astype(BF)

        # per-row avg-pool weights wrm/len, one column per clause-within-tile
        lens = np.maximum(wrm_f.reshape(NCL, LC).sum(1), 1.0)
        aux_h = np.concatenate([
            np.tile((1.0 / lens)[None, :], (128, 1)),
            f32("dec_w").reshape(HC, 128, NL).transpose(1, 0, 2).reshape(128, HC * NL),
        ], axis=1).astype(BF)

        adjc = adj[sl].reshape(NCL, LC, LC)
        adjT = np.zeros((RT, 128, 128), dtype=np.float32)
        for q in range(NCL):
            r, ii = q // 4, q % 4
            adjT[r, 32 * ii : 32 * ii + 32, 32 * ii : 32 * ii + 32] = adjc[q].T
        adjT = adjT.transpose(1, 0, 2).reshape(128, RT * 128).astype(BF)

        d = dict(
            xtg=np.ascontiguousarray(xtg_h),
            wrmb=wrm_f.astype(BF),
            aux=np.ascontiguousarray(aux_h),
            adjt=np.ascontiguousarray(adjT),
            wg=wg_pk, wa=wa_pk, wb=wb_pk,
            smallpk=smallpk, biasbc=biasbc,
            percl=np.ascontiguousarray(percl),
        )
        in_maps.append(d)
    return in_maps


def run_spmd(inputs, trace=False):
    nc = _get_program()
    in_maps = shard_inputs(inputs)
    kw = {}
    if trace:
        import types
        from trn_agent_boot.trn_boot import _ntff_profile_via_ctypes
        mod = types.ModuleType("antenv.axon_hooks")
        hook = _ntff_profile_via_ctypes("/opt/axon/libaxon_pjrt.so")
        mod.get_axon_ntff_profile_hook = lambda: hook
        mod.set_axon_ntff_profile_hook = lambda h: None
        sys.modules["antenv.axon_hooks"] = mod
        bass_utils.upload_artifacts = lambda tmpdir: "local://" + tmpdir
        kw["trace"] = True
    res = bass_utils.run_bass_kernel_spmd(nc, in_maps, core_ids=list(range(NCORES)), **kw)
    return res


def kernel(**inputs):
    res = run_spmd(inputs)
    cn = np.asarray(inputs["clause_num_mask"], dtype=np.float32)
    num = 0.0
    den = 0.0
    for i in range(NCORES):
        kl = np.asarray(res.results[i]["out"], dtype=np.float64)
        cnm = cn[BB * i : BB * i + BB].reshape(NCL).astype(np.float64)
        num += float((kl * cnm).sum())
        den += float(cnm.sum())
    loss = (num / NL) / den
    return np.asarray(loss, dtype=np.float32)


# revision 124
# speedup vs baseline: 1.0475x; 1.0475x over previous
"""Trainium2 Bass kernel for nn_MESGM_15857019256842.

Data-parallel over batch: 16 batches -> 8 cores x 2 batches.
Per core: gather clause tokens (indirect DMA, bf16), 2 GCN layers with
pooling fused into the per-group loop, projection, 8-head self-attention
over 2x32 clauses, FFN, label decoder, soft-label KL loss.
Each core emits (sum kl*mask, sum mask); host combines.

Host-side prep packs all weights into bf16 DRAM arrays laid out exactly
as the SBUF tiles want them (few large DMAs), pre-builds the transposed
block-diagonal adjacency, and pre-casts the encoder output to bf16.
Attention/projection weights prefetch during the GCN phase; FFN weights
prefetch during attention.
"""
import sys
sys.path.insert(0, '/opt/trn_rl_repo')
import numpy as np

from concourse import bass, mybir, tile
from concourse import bass_utils
from concourse.masks import make_identity
from concourse.vector_clock import ScopedClock

F32 = mybir.dt.float32
BF16 = mybir.dt.bfloat16
I32 = mybir.dt.int32
AF = mybir.ActivationFunctionType
AX = mybir.AxisListType
ALU = mybir.AluOpType

B, S, H, M, LC, NL, II, NH, DH = 16, 512, 768, 32, 32, 7, 3072, 8, 96
NCORES = 8
BB = B // NCORES          # 2 batches per core
NCL = BB * M              # 64 clauses per core
NROW = NCL * LC           # 2048 clause-token rows per core
RT = NROW // 128          # 16 row tiles
HC = H // 128             # 6 H chunks
IC = II // 128            # 24 intermediate chunks
LN_EPS = 1e-12
SQD = float(np.sqrt(DH))

# column layout of the packed small-constants tile [128, 100] f32
SP_GB1, SP_GB2, SP_PJB, SP_QB, SP_KB, SP_INTB, SP_DW = 0, 6, 12, 18, 26, 34, 58
# column layout of the broadcast-bias pack [5383] f32
BB_AOB, BB_VB, BB_OUTB, BB_L1G, BB_L1B, BB_L2G, BB_L2B, BB_DECB = (
    0, 768, 1536, 2304, 3072, 3840, 4608, 5376)
NBB = 5383

_MAX_WAITS = 1


def _patched_drain_and_barrier(self, tick_clock, wait_clock):
    nc = self.nc
    drain_inst = nc.sync.drain()
    wait_clock.add_sem_waits(
        drain_inst.ins, ScopedClock({None: tick_clock.global_clock})
    )
    si = drain_inst.ins.sync_info
    waits = list(si.on_wait or [])
    if len(waits) > _MAX_WAITS:
        si.on_wait = waits[:_MAX_WAITS]
        rest = waits[_MAX_WAITS:]
        for i in range(0, len(rest), _MAX_WAITS):
            nop = nc.sync.nop(nofuse=True)
            nop.ins.sync_info = mybir.SyncInfo(
                on_wait=rest[i : i + _MAX_WAITS], on_update=[]
            )
    nc.all_engine_barrier()
    assert self.sems is not None
    popped = nc._tile_sem_poison_stack.pop()
    assert popped is self._sem_poison
    nc.clear_and_free_semaphores(list(self.sems.allocated().values()))
    nc.all_engine_barrier()


tile.TileContext._drain_and_barrier = _patched_drain_and_barrier


def legalize_waits(nc, limit=1):
    """TRN2 instructions carry at most one sem wait; hoist extras onto nops."""
    nfix = 0
    for blk in nc.main_func.blocks:
        insts = list(blk.instructions)
        pos = 0
        for inst in insts:
            si = inst.sync_info
            waits = list(si.on_wait) if si is not None and si.on_wait else []
            if len(waits) > limit:
                si.on_wait = waits[-limit:]
                rest = waits[:-limit]
                eng = nc.engines[inst.engine]
                for j in range(0, len(rest), limit):
                    nop = eng.nop(nofuse=True)
                    nop.ins.sync_info = mybir.SyncInfo(
                        on_wait=rest[j : j + limit], on_update=[]
                    )
                    src_blk = nc.cur_bb.bb
                    popped = src_blk.instructions.pop()
                    assert popped.name == nop.ins.name
                    blk.instructions.insert(pos, nop.ins)
                    pos += 1
                nfix += 1
            pos += 1
    return nfix


def build_program():
    nc = bass.Bass(trn_type="TRN2")

    # ---- DRAM I/O (everything pre-packed on host) ------------------------
    # xtg holds the pre-gathered masked tokens already transposed into
    # [128 h-part, group, h-chunk, 512 rows] tile layout
    xtg = nc.dram_tensor("xtg", [128, RT * H], BF16, kind="ExternalInput")
    wrmb = nc.dram_tensor("wrmb", [NROW], BF16, kind="ExternalInput")
    aux = nc.dram_tensor("aux", [128, NCL + HC * NL], BF16,
                         kind="ExternalInput")
    adjt = nc.dram_tensor("adjt", [128, RT * 128], BF16, kind="ExternalInput")
    wg = nc.dram_tensor("wg", [128, 2 * HC * H], BF16, kind="ExternalInput")
    wa = nc.dram_tensor("wa", [128, (24 + 4 * HC) * H], BF16, kind="ExternalInput")
    wb = nc.dram_tensor("wb", [128, HC * II + IC * H], BF16, kind="ExternalInput")
    smallpk = nc.dram_tensor("smallpk", [128, 100], F32, kind="ExternalInput")
    biasbc = nc.dram_tensor("biasbc", [NBB], F32, kind="ExternalInput")
    percl = nc.dram_tensor("percl", [NCL, 520], F32, kind="ExternalInput")
    out_d = nc.dram_tensor("out", [2], F32, kind="ExternalOutput")

    with tile.TileContext(nc) as tc:
        _body(nc, tc, xtg, wrmb, aux, adjt, wg, wa, wb, smallpk,
              biasbc, percl, out_d)

    nfix = legalize_waits(nc)
    return nc, nfix


def _body(nc, tc, xtg, wrmb, aux, adjt, wg, wa, wb, smallpk,
          biasbc, percl, out_d):
    from contextlib import ExitStack
    ctx = ExitStack()
    with ctx:
        pp = ctx.enter_context(tc.tile_pool(name="persist", bufs=1))

        ident = pp.tile([128, 128], F32, tag="ident")
        make_identity(nc, ident[:])
        ident_b = pp.tile([128, 128], BF16, tag="identb")
        nc.vector.tensor_copy(out=ident_b[:], in_=ident[:])

        sp = pp.tile([128, 100], F32, tag="smallpk")
        nc.sync.dma_start(out=sp[:], in_=smallpk[:, :])

        # warm the natural_log_exp table set (covers every transcendental
        # we use except gelu) while the first DMAs stream
        warm = pp.tile([128, 1], F32, tag="actwarm")
        nc.vector.memset(warm[:], 0.5)
        nc.scalar.activation(out=warm[:], in_=warm[:], func=AF.Ln)
        nc.scalar.activation(out=warm[:], in_=warm[:], func=AF.Exp)
        nc.scalar.mul(out=sp[:DH, SP_QB : SP_QB + NH],
                      in_=sp[:DH, SP_QB : SP_QB + NH], mul=1.0 / SQD)

        PT = pp.tile([128, 24, NCL], BF16, tag="PT")
        eps_t = pp.tile([NCL, 1], F32, tag="epst")
        nc.vector.memset(eps_t[:], LN_EPS)


        bb_t = pp.tile([NCL, NBB], F32, tag="biasbc")
        pcl = pp.tile([NCL, 520], F32, tag="percl")
        amask8 = pcl[:, 0:512].rearrange("p (h n) -> p h n", h=NH)
        tgt_sb = pcl[:, 512:519]
        cnm_pp = pcl[:, 519:520]
        # t*ln(t) (and its row-sum) for the KL tail, computed in phase 1
        a1_early = pp.tile([NCL, NL], F32, tag="a1early")
        s_a1e = pp.tile([NCL, 1], F32, tag="sa1e")
        pair = pp.tile([NCL, 2], F32, tag="pair")
        ones_t = pp.tile([NCL, 1], F32, tag="onest")
        nc.vector.memset(ones_t[:], 1.0)
        # replicated 1/len per clause + bf16 decoder weights (tiny)
        aux_t = pp.tile([128, NCL + HC * NL], BF16, tag="aux")
        lens_r = aux_t[:, 0:NCL]

        # attention/projection weights tile (resident through phase 2).
        # DMAs are issued on the scalar HWDGE queue after GCN group 0 so
        # they don't compete with the critical-path loads early on.
        WAC = (24 + 4 * HC) * H
        wa_t = ctx.enter_context(tc.tile_pool(name="wa", bufs=1)).tile(
            [128, WAC], BF16, tag="wa")

        def issue_wa_loads():
            # delayed behind a virtual timestamp so these (large, non-
            # critical) loads don't steal DMA bandwidth from the phase-1
            # critical path (the scheduler otherwise hoists dep-free DMAs
            # to the very front)
            with tc.tile_wait_until(0.015):
                nc.sync.dma_start(out=bb_t[:],
                                  in_=bass.AP(tensor=biasbc, offset=0,
                                              ap=[[0, NCL], [1, NBB]]))
                nc.sync.dma_start(out=pcl[:], in_=percl[:, :])
                for j in range(6):
                    c0, c1 = j * (WAC // 6), (j + 1) * (WAC // 6)
                    nc.sync.dma_start(out=wa_t[:, c0:c1],
                                      in_=bass.AP(tensor=wa, offset=c0,
                                                  ap=[[WAC, 128], [1, c1 - c0]]))
                # KL's t*ln(t) term is independent of everything else
                lnt = pscr.tile([NCL, NL], F32, tag="lnt")
                nc.scalar.activation(out=lnt[:], in_=tgt_sb, func=AF.Ln)
                nc.vector.tensor_tensor(out=a1_early[:], in0=tgt_sb, in1=lnt[:],
                                        op=ALU.mult)
                nc.vector.reduce_sum(out=s_a1e[:], in_=a1_early[:], axis=AX.X)
                nc.vector.tensor_copy(out=pair[:, 1:2], in_=cnm_pp)

        projw = wa_t[:, 0 : 24 * H]
        qw = wa_t[:, 24 * H : 30 * H]
        kw = wa_t[:, 30 * H : 36 * H]
        vw = wa_t[:, 36 * H : 42 * H]
        aow = wa_t[:, 42 * H : 48 * H]

        # proj psum lives across layer 2 so its X-column accumulation
        # steps can interleave with the GCN as PE gap-filler
        pjps = ctx.enter_context(tc.tile_pool(name="pjps", bufs=1, space="PSUM"))
        pcs = pjps.tile([128, HC, NCL], F32, tag="pj")
        korder = (list(range(0, 6)) + list(range(12, 18))
                  + list(range(6, 12)) + list(range(18, 24)))

        def proj_steps(kis):
            for ki in kis:
                k = korder[ki]
                for m in range(HC):
                    nc.tensor.matmul(
                        out=pcs[:, m, :],
                        lhsT=projw[:, k * H + m * 128 : k * H + m * 128 + 128],
                        rhs=PT[:, k, :], start=(ki == 0), stop=(ki == 23))

        # =================== phase 1: gather + GCN + pooling ==============
        ph1 = ExitStack()
        p1p = ph1.enter_context(tc.tile_pool(name="p1misc", bufs=1))
        xmt = ph1.enter_context(tc.tile_pool(name="xmt", bufs=3))
        wg_t = p1p.tile([128, 2 * HC * H], BF16, tag="wg")
        adjT = p1p.tile([128, RT, 128], BF16, tag="adjT")
        wrm_bcb = p1p.tile([128, NROW], BF16, tag="wrmbcb")

        # hand-ordered sync-queue loads: token group 0 first, then gc1,
        # remaining token groups interleaved with the rest.
        xmts = []
        for g in range(4):
            xmts.append(xmt.tile([128, HC, 512], BF16, tag="xmt", name=f"xmt{g}"))

        def xmt_load(g):
            nc.sync.dma_start(out=xmts[g][:],
                              in_=bass.AP(tensor=xtg, offset=g * HC * 512,
                                          ap=[[RT * H, 128], [1, HC * 512]]))

        xmt_load(0)
        nc.sync.dma_start(out=wg_t[:, 0 : HC * H],
                          in_=bass.AP(tensor=wg, offset=0,
                                      ap=[[2 * HC * H, 128], [1, HC * H]]))
        xmt_load(1)
        xmt_load(2)
        nc.sync.dma_start(out=adjT[:], in_=adjt[:, :])
        nc.sync.dma_start(out=wg_t[:, HC * H : 2 * HC * H],
                          in_=bass.AP(tensor=wg, offset=HC * H,
                                      ap=[[2 * HC * H, 128], [1, HC * H]]))
        nc.sync.dma_start(out=wrm_bcb[:],
                          in_=bass.AP(tensor=wrmb, offset=0,
                                      ap=[[0, 128], [1, NROW]]))
        nc.sync.dma_start(out=aux_t[:], in_=aux[:, :])
        xmt_load(3)

        big = ph1.enter_context(tc.tile_pool(name="big", bufs=1))
        H1T = big.tile([128, HC, NROW], BF16, tag="H1T")
        ynp = ph1.enter_context(tc.tile_pool(name="ynp", bufs=2))
        h2p = ph1.enter_context(tc.tile_pool(name="h2p", bufs=2))
        pscr = ph1.enter_context(tc.tile_pool(name="pscr", bufs=4))
        gps1 = ph1.enter_context(tc.tile_pool(name="gps1", bufs=3, space="PSUM"))
        gps2 = ph1.enter_context(tc.tile_pool(name="gps2", bufs=2, space="PSUM"))
        zps = ph1.enter_context(tc.tile_pool(name="zps", bufs=2, space="PSUM"))

        def y_block(XT, xoff, wofs, g, tag):
            """XT[:, c, xoff:xoff+512] @ W -> 4 row tiles of y, bf16 SBUF."""
            yns = []
            for rr in range(4):
                p1 = gps1.tile([128, 512], F32, tag="y1", name=f"y1_{tag}{g}_{rr}")
                p2 = gps2.tile([128, 256], F32, tag="y2", name=f"y2_{tag}{g}_{rr}")
                for c in range(HC):
                    lhs = XT[:, c, xoff + rr * 128 : xoff + rr * 128 + 128]
                    nc.tensor.matmul(out=p1[:], lhsT=lhs,
                                     rhs=wg_t[:, wofs + c * H : wofs + c * H + 512],
                                     start=(c == 0), stop=(c == HC - 1))
                    nc.tensor.matmul(out=p2[:], lhsT=lhs,
                                     rhs=wg_t[:, wofs + c * H + 512 : wofs + c * H + 768],
                                     start=(c == 0), stop=(c == HC - 1))
                yr = ynp.tile([128, H], BF16, tag=f"yn{rr}", name=f"yn_{tag}{g}_{rr}")
                nc.scalar.copy(out=yr[:, 0:512], in_=p1[:])
                nc.vector.tensor_copy(out=yr[:, 512:768], in_=p2[:])
                yns.append(yr)
            return yns

        def z_block(yns, g, bcol, HT, hoff, tag):
            """adj @ y -> relu(. + b) into HT[:, c, hoff:hoff+512] (bf16)."""
            for c in range(HC):
                zp = zps.tile([128, 512], F32, tag="z", name=f"z_{tag}{g}_{c}")
                for rr in range(4):
                    nc.tensor.matmul(
                        out=zp[:, rr * 128 : rr * 128 + 128],
                        lhsT=yns[rr][:, c * 128 : c * 128 + 128],
                        rhs=adjT[:, 4 * g + rr, :],
                        start=True, stop=True,
                    )
                nc.scalar.activation(
                    out=HT[:, c, hoff : hoff + 512], in_=zp[:],
                    func=AF.Relu, bias=sp[:, bcol + c : bcol + c + 1], scale=1.0,
                )

        def pool_group(XT, xoff, g, mcol, acol):
            """max/avg pool 512 cols (16 clauses) of XT into PT."""
            for c in range(HC):
                v = XT[:, c, xoff : xoff + 512].rearrange("p (n l) -> p n l", l=LC)
                nc.vector.reduce_max(out=PT[:, mcol + c, 16 * g : 16 * g + 16],
                                     in_=v, axis=AX.X)
                st = pscr.tile([128, 16], F32, tag="pst", name=f"pst{mcol}_{g}_{c}")
                nc.vector.reduce_sum(out=st[:], in_=v, axis=AX.X)
                nc.gpsimd.tensor_tensor(
                    out=PT[:, acol + c, 16 * g : 16 * g + 16], in0=st[:],
                    in1=lens_r[:, 16 * g : 16 * g + 16], op=ALU.mult)

        # ---- layer 1 -----------------------------------------------------
        for g in range(4):
            if g == 1:
                issue_wa_loads()
            XmT = xmts[g]
            pool_group(XmT, 0, g, 0, 12)
            yns = y_block(XmT, 0, 0, g, "l1")
            z_block(yns, g, SP_GB1, H1T, g * 512, "l1")

        # ---- layer 2 (H2 pooled on the fly, never materialized) ----------
        # proj accumulation over the X-pool columns, emitted before ANY
        # l2 pool write so the access history carries no false deps; the
        # PE works through these during l2's vector-bound stretches
        proj_steps(range(12))
        for g in range(4):
            yns = y_block(H1T, g * 512, HC * H, g, "l2")
            h2g = h2p.tile([128, HC, 512], BF16, tag="h2g", name=f"h2g{g}")
            z_block(yns, g, SP_GB2, h2g, 0, "l2")
            for c in range(HC):
                nc.gpsimd.tensor_tensor(out=h2g[:, c, :], in0=h2g[:, c, :],
                                        in1=wrm_bcb[:, g * 512 : g * 512 + 512],
                                        op=ALU.mult)
            pool_group(h2g, 0, g, 6, 18)

        ph1.close()

        # =================== phase 2: proj + attention + FFN ==============
        ph2 = ExitStack()
        wbp = ph2.enter_context(tc.tile_pool(name="wbp", bufs=1))
        WBC = HC * II + IC * H
        wb_t = wbp.tile([128, WBC], BF16, tag="wb")
        for j in range(6):
            c0, c1 = j * (WBC // 6), (j + 1) * (WBC // 6)
            nc.sync.dma_start(out=wb_t[:, c0:c1],
                              in_=bass.AP(tensor=wb, offset=c0,
                                          ap=[[WBC, 128], [1, c1 - c0]]))
        intw = wb_t[:, 0 : HC * II]
        outw = wb_t[:, HC * II :]

        at = ph2.enter_context(tc.tile_pool(name="attn", bufs=1))
        sc = ph2.enter_context(tc.tile_pool(name="scr", bufs=1))

        # projection: finish the accumulation (H2-pool columns), then relu
        cvT = at.tile([128, HC, NCL], BF16, tag="cvT")
        proj_steps(range(12, 24))
        for m in range(HC):
            nc.scalar.activation(out=cvT[:, m, :], in_=pcs[:, m, :], func=AF.Relu,
                                 bias=sp[:, SP_PJB + m : SP_PJB + m + 1], scale=1.0)

        # cv natural + ao_b (residual base)
        cv_pa = at.tile([NCL, H], F32, tag="cvpa")
        with tc.tile_pool(name="cvt2", bufs=3, space="PSUM") as cvt2:
            for c in range(HC):
                ps = cvt2.tile([NCL, 128], BF16, tag="cvn", name=f"cvn{c}")
                nc.tensor.transpose(out=ps[:], in_=cvT[:, c, :], identity=ident_b[:])
                nc.vector.tensor_tensor(out=cv_pa[:, c * 128 : c * 128 + 128], in0=ps[:],
                                        in1=bb_t[:, BB_AOB + c * 128 : BB_AOB + c * 128 + 128],
                                        op=ALU.add)

        # attention
        QT = at.tile([DH, NH, NCL], BF16, tag="QT")
        KT = at.tile([DH, NH, NCL], BF16, tag="KT")
        Vn = at.tile([NCL, H], BF16, tag="Vn")
        ctx_nat = at.tile([NCL, H], BF16, tag="ctxn")
        att8 = at.tile([NCL, NH, NCL], BF16, tag="att8")
        sums_t = at.tile([NCL, NH], F32, tag="sums")
        recip_t = at.tile([NCL, NH], F32, tag="recip")
        s2 = at.tile([NCL, NH, NCL], F32, tag="s2")
        negmax = at.tile([NCL, NH], F32, tag="negmax")

        with tc.tile_pool(name="qkps", bufs=2, space="PSUM") as qkps, \
             tc.tile_pool(name="vps", bufs=1, space="PSUM") as vps, \
             tc.tile_pool(name="scps", bufs=1, space="PSUM") as scps:
            psq = qkps.tile([DH, NH * NCL], F32, tag="qk", name="psq")
            for h in range(NH):
                for c in range(HC):
                    nc.tensor.matmul(out=psq[:, h * NCL : h * NCL + NCL],
                                     lhsT=qw[:, c * H + h * DH : c * H + h * DH + DH],
                                     rhs=cvT[:, c, :], start=(c == 0), stop=(c == HC - 1))
            for h in range(NH):
                nc.scalar.activation(out=QT[:, h, :], in_=psq[:, h * NCL : h * NCL + NCL],
                                     func=AF.Identity,
                                     bias=sp[:DH, SP_QB + h : SP_QB + h + 1], scale=1.0 / SQD)
            psk = qkps.tile([DH, NH * NCL], F32, tag="qk", name="psk")
            for h in range(NH):
                for c in range(HC):
                    nc.tensor.matmul(out=psk[:, h * NCL : h * NCL + NCL],
                                     lhsT=kw[:, c * H + h * DH : c * H + h * DH + DH],
                                     rhs=cvT[:, c, :], start=(c == 0), stop=(c == HC - 1))
            for h in range(NH):
                nc.scalar.activation(out=KT[:, h, :], in_=psk[:, h * NCL : h * NCL + NCL],
                                     func=AF.Identity,
                                     bias=sp[:DH, SP_KB + h : SP_KB + h + 1], scale=1.0)

            pv1 = vps.tile([NCL, 512], F32, tag="v1")
            pv2 = vps.tile([NCL, 256], F32, tag="v2")
            for c in range(HC):
                nc.tensor.matmul(out=pv1[:], lhsT=cvT[:, c, :],
                                 rhs=vw[:, c * H : c * H + 512],
                                 start=(c == 0), stop=(c == HC - 1))
                nc.tensor.matmul(out=pv2[:], lhsT=cvT[:, c, :],
                                 rhs=vw[:, c * H + 512 : c * H + 768],
                                 start=(c == 0), stop=(c == HC - 1))
            nc.vector.tensor_tensor(out=Vn[:, 0:512], in0=pv1[:],
                                    in1=bb_t[:, BB_VB : BB_VB + 512], op=ALU.add)
            nc.vector.tensor_tensor(out=Vn[:, 512:768], in0=pv2[:],
                                    in1=bb_t[:, BB_VB + 512 : BB_VB + 768], op=ALU.add)

            pss = scps.tile([NCL, NH * NCL], F32, tag="scores")
            for h in range(NH):
                nc.tensor.matmul(out=pss[:, h * NCL : h * NCL + NCL], lhsT=QT[:, h, :],
                                 rhs=KT[:, h, :], start=True, stop=True)
            nc.vector.tensor_tensor(out=s2[:], in0=pss[:].rearrange("p (h n) -> p h n", h=NH),
                                    in1=amask8, op=ALU.add)
        nc.vector.tensor_reduce(out=negmax[:], in_=s2[:], axis=AX.X, op=ALU.max,
                                negate=True)
        for h in range(NH):
            nc.scalar.activation(
                out=att8[:, h, :], in_=s2[:, h, :], func=AF.Exp,
                bias=negmax[:, h : h + 1], scale=1.0,
                accum_out=sums_t[:, h : h + 1],
            )
        nc.vector.reciprocal(out=recip_t[:], in_=sums_t[:])
        with tc.tile_pool(name="ctps", bufs=3, space="PSUM") as ctps, \
             tc.tile_pool(name="atts", bufs=3) as atts:
            for h in range(NH):
                pst = ctps.tile([NCL, NCL], BF16, tag="attT", name=f"attT{h}")
                nc.tensor.transpose(out=pst[:], in_=att8[:, h, :], identity=ident_b[:64, :64])
                asb = atts.tile([NCL, NCL], BF16, tag="attTs", name=f"attTs{h}")
                nc.vector.tensor_copy(out=asb[:], in_=pst[:])
                pctx = ctps.tile([NCL, DH], F32, tag="ctx", name=f"ctx{h}")
                nc.tensor.matmul(out=pctx[:], lhsT=asb[:], rhs=Vn[:, h * DH : h * DH + DH],
                                 start=True, stop=True)
                nc.scalar.mul(out=ctx_nat[:, h * DH : h * DH + DH], in_=pctx[:],
                              mul=recip_t[:, h : h + 1])

        def transpose_to(src_nat, dstT, dt=F32):
            idn = ident if dt == F32 else ident_b
            with tc.tile_pool(name="trp", bufs=3, space="PSUM") as trp:
                for c in range(HC):
                    ps = trp.tile([128, NCL], dt, tag="trt", name=f"trt{c}")
                    nc.tensor.transpose(out=ps[:], in_=src_nat[:, c * 128 : c * 128 + 128],
                                        identity=idn[:64, :64])
                    if c % 2 == 0:
                        nc.vector.tensor_copy(out=dstT[:, c, :], in_=ps[:])
                    else:
                        nc.scalar.copy(out=dstT[:, c, :], in_=ps[:])

        def layer_norm(x_nat, gcol, bcol, y_nat):
            with tc.tile_pool(name="lnp", bufs=1) as lnp:
                stats = lnp.tile([NCL, 3, 6], F32, tag="lnstats")
                for i in range(3):
                    nc.vector.bn_stats(out=stats[:, i, :], in_=x_nat[:, i * 256 : i * 256 + 256])
                mv = lnp.tile([NCL, 2], F32, tag="lnmv")
                nc.vector.bn_aggr(out=mv[:], in_=stats[:])
                # rstd = exp(-0.5*ln(var+eps)) — stays in the natlog_exp
                # table set (a Sqrt would force a table switch)
                lv = lnp.tile([NCL, 1], F32, tag="lnlv")
                nc.scalar.activation(out=lv[:], in_=mv[:, 1:2], func=AF.Ln, bias=eps_t[:, :1], scale=1.0)
                rstd = lnp.tile([NCL, 1], F32, tag="lnrstd")
                nc.scalar.activation(out=rstd[:], in_=lv[:], func=AF.Exp, scale=-0.5)
                # normalize in place (x_nat is dead after), elementwise
                # tail split across DVE and GpSimd
                nc.vector.tensor_scalar(out=x_nat[:, 0:512], in0=x_nat[:, 0:512],
                                        scalar1=mv[:, 0:1], scalar2=rstd[:, :1],
                                        op0=ALU.subtract, op1=ALU.mult)
                nc.gpsimd.tensor_scalar(out=x_nat[:, 512:768], in0=x_nat[:, 512:768],
                                        scalar1=mv[:, 0:1], scalar2=rstd[:, :1],
                                        op0=ALU.subtract, op1=ALU.mult)
                nc.vector.tensor_tensor(out=x_nat[:, 0:512], in0=x_nat[:, 0:512],
                                        in1=bb_t[:, gcol : gcol + 512], op=ALU.mult)
                nc.gpsimd.tensor_tensor(out=x_nat[:, 512:768], in0=x_nat[:, 512:768],
                                        in1=bb_t[:, gcol + 512 : gcol + H], op=ALU.mult)
                nc.vector.tensor_tensor(out=y_nat[:, 0:512], in0=x_nat[:, 0:512],
                                        in1=bb_t[:, bcol : bcol + 512], op=ALU.add)
                nc.gpsimd.tensor_tensor(out=y_nat[:, 512:768], in0=x_nat[:, 512:768],
                                        in1=bb_t[:, bcol + 512 : bcol + H], op=ALU.add)

        # ao proj + residual + LN1
        ctxT = at.tile([128, HC, NCL], BF16, tag="ctxT")
        transpose_to(ctx_nat, ctxT, dt=BF16)
        attn_out = at.tile([NCL, H], F32, tag="attnout")
        with tc.tile_pool(name="aops", bufs=1, space="PSUM") as aops:
            pa1 = aops.tile([NCL, 512], F32, tag="ao1")
            pa2 = aops.tile([NCL, 256], F32, tag="ao2")
            for c in range(HC):
                nc.tensor.matmul(out=pa1[:], lhsT=ctxT[:, c, :],
                                 rhs=aow[:, c * H : c * H + 512],
                                 start=(c == 0), stop=(c == HC - 1))
                nc.tensor.matmul(out=pa2[:], lhsT=ctxT[:, c, :],
                                 rhs=aow[:, c * H + 512 : c * H + 768],
                                 start=(c == 0), stop=(c == HC - 1))
            ln_in = sc.tile([NCL, H], F32, tag="lnin1")
            nc.vector.tensor_tensor(out=ln_in[:, 0:512], in0=pa1[:], in1=cv_pa[:, 0:512], op=ALU.add)
            nc.vector.tensor_tensor(out=ln_in[:, 512:768], in0=pa2[:], in1=cv_pa[:, 512:768], op=ALU.add)
            layer_norm(ln_in, BB_L1G, BB_L1B, attn_out)
        # dummy gelu: pulls the gelu table load off the critical path,
        # overlapping it with the aoT transposes + FFN1 matmuls
        nc.scalar.activation(out=warm[:], in_=warm[:], func=AF.Gelu)

        # FFN1 computed transposed: interT[:, j, :] = gelu(int_w[:, j]^T @ ao + b_j)
        aoT = at.tile([128, HC, NCL], BF16, tag="aoT")
        transpose_to(attn_out, aoT)
        interT = at.tile([128, IC, NCL], BF16, tag="interT")
        out_nat = at.tile([NCL, H], BF16, tag="outnat")
        with tc.tile_pool(name="fps", bufs=1, space="PSUM") as fps, \
             tc.tile_pool(name="ops", bufs=1, space="PSUM") as ops:
            psf = [fps.tile([128, 8, NCL], F32, tag=f"fi{n}", name=f"fi{n}") for n in range(3)]
            for j in range(IC):
                for c in range(HC):
                    nc.tensor.matmul(out=psf[j // 8][:, j % 8, :],
                                     lhsT=intw[:, c * II + j * 128 : c * II + j * 128 + 128],
                                     rhs=aoT[:, c, :], start=(c == 0), stop=(c == HC - 1))
            for j in range(IC):
                nc.scalar.activation(out=interT[:, j, :], in_=psf[j // 8][:, j % 8, :],
                                     func=AF.Gelu,
                                     bias=sp[:, SP_INTB + j : SP_INTB + j + 1], scale=1.0)
            # dummy ln: reloads natlog_exp during the FFN2 matmuls so
            # LN2/KL don't pay the table switch
            nc.scalar.activation(out=warm[:], in_=warm[:], func=AF.Ln)

            po1 = ops.tile([NCL, 512], F32, tag="o1")
            po2 = ops.tile([NCL, 256], F32, tag="o2")
            for cc in range(IC):
                nc.tensor.matmul(out=po1[:], lhsT=interT[:, cc, :],
                                 rhs=outw[:, cc * H : cc * H + 512],
                                 start=(cc == 0), stop=(cc == IC - 1))
                nc.tensor.matmul(out=po2[:], lhsT=interT[:, cc, :],
                                 rhs=outw[:, cc * H + 512 : cc * H + 768],
                                 start=(cc == 0), stop=(cc == IC - 1))
            ln_in2 = sc.tile([NCL, H], F32, tag="lnin2")
            nc.vector.tensor_tensor(out=ln_in2[:, 0:512], in0=po1[:], in1=attn_out[:, 0:512], op=ALU.add)
            nc.vector.tensor_tensor(out=ln_in2[:, 512:768], in0=po2[:], in1=attn_out[:, 512:768], op=ALU.add)
            nc.vector.tensor_tensor(out=ln_in2[:], in0=ln_in2[:],
                                    in1=bb_t[:, BB_OUTB : BB_OUTB + H], op=ALU.add)
            layer_norm(ln_in2, BB_L2G, BB_L2B, out_nat)

        # decoder + KL
        outT = at.tile([128, HC, NCL], BF16, tag="outT")
        transpose_to(out_nat, outT, dt=BF16)
        dwb = aux_t[:, NCL : NCL + HC * NL].rearrange("p (c l) -> p c l", l=NL)

        with tc.tile_pool(name="klps", bufs=1, space="PSUM") as klps, \
             tc.tile_pool(name="klsc", bufs=1) as klsc:
            pd = klps.tile([NCL, NL], F32, tag="pred")
            for c in range(HC):
                nc.tensor.matmul(out=pd[:], lhsT=outT[:, c, :], rhs=dwb[:, c, :],
                                 start=(c == 0), stop=(c == HC - 1))
            pred = klsc.tile([NCL, NL], F32, tag="pred_sb")
            nc.vector.tensor_tensor(out=pred[:], in0=pd[:],
                                    in1=bb_t[:, BB_DECB : BB_DECB + NL], op=ALU.add)
            negm = klsc.tile([NCL, 1], F32, tag="negm")
            nc.vector.tensor_reduce(out=negm[:], in_=pred[:], axis=AX.X, op=ALU.max, negate=True)
            # kl = sum(t*ln t) - sum(t*pred) - (lnS - negm)   [sum(t) == 1]
            esc = klsc.tile([NCL, NL], F32, tag="esc")
            ssum = klsc.tile([NCL, 1], F32, tag="ssum")
            nc.scalar.activation(out=esc[:], in_=pred[:], func=AF.Exp,
                                 bias=negm[:, :1], scale=1.0, accum_out=ssum[:, :1])
            lnS = klsc.tile([NCL, 1], F32, tag="lnS")
            nc.scalar.activation(out=lnS[:], in_=ssum[:], func=AF.Ln)
            c1 = klsc.tile([NCL, 1], F32, tag="c1")
            nc.scalar.activation(out=c1[:], in_=lnS[:], func=AF.Identity,
                                 bias=negm[:, :1], scale=-1.0)
            tp = klsc.tile([NCL, NL], F32, tag="tp")
            nc.vector.tensor_tensor(out=tp[:], in0=tgt_sb, in1=pred[:], op=ALU.mult)
            stp = klsc.tile([NCL, 1], F32, tag="stp")
            nc.vector.reduce_sum(out=stp[:], in_=tp[:], axis=AX.X)
            kd = klsc.tile([NCL, 1], F32, tag="kd")
            nc.vector.tensor_tensor(out=kd[:], in0=s_a1e[:], in1=stp[:], op=ALU.subtract)
            kl = klsc.tile([NCL, 1], F32, tag="kl")
            nc.vector.tensor_tensor(out=kl[:], in0=kd[:], in1=c1[:], op=ALU.subtract)
            nc.vector.tensor_tensor(out=pair[:, 0:1], in0=kl[:], in1=cnm_pp, op=ALU.mult)
            pf = klps.tile([2, 1], F32, tag="fin_ps")
            nc.tensor.matmul(out=pf[:], lhsT=pair[:], rhs=ones_t[:], start=True, stop=True)
            fin_sb = klsc.tile([2, 1], F32, tag="fin")
            nc.vector.tensor_copy(out=fin_sb[:], in_=pf[:])
            nc.sync.dma_start(out=out_d[:, None], in_=fin_sb[:])
        ph2.close()


_CACHE = {}


def _get_program():
    if "nc" not in _CACHE:
        nc, nfix = build_program()
        _CACHE["nc"] = nc
    return _CACHE["nc"]


def _chunk_pack(w_, nchunk):
    """[nchunk*128, cols] -> [128, nchunk*cols] with chunk k at cols k*cols."""
    cols = w_.shape[1]
    return np.ascontiguousarray(
        w_.reshape(nchunk, 128, cols).transpose(1, 0, 2).reshape(128, nchunk * cols))


def shard_inputs(inputs):
    import ml_dtypes
    BF = ml_dtypes.bfloat16
    enc = np.asarray(inputs["encoder_hs"], dtype=np.float32)
    wr = np.asarray(inputs["word_recovery"], dtype=np.int32)
    wm = np.asarray(inputs["word_recovery_mask"], dtype=np.int32)
    cn = np.asarray(inputs["clause_num_mask"], dtype=np.int32)
    adj = np.asarray(inputs["adj_matrix"], dtype=np.float32)
    tl = np.asarray(inputs["target_labels"], dtype=np.float32)

    f32 = lambda k: np.asarray(inputs[k], dtype=np.float32)

    # ---- shared weight packs (identical across cores) --------------------
    wg_pk = np.concatenate([
        _chunk_pack(f32("gc1_w"), HC), _chunk_pack(f32("gc2_w"), HC)], axis=1).astype(BF)
    wa_pk = np.concatenate([
        _chunk_pack(f32("proj_w"), 24),
        _chunk_pack(f32("q_w"), HC), _chunk_pack(f32("k_w"), HC),
        _chunk_pack(f32("v_w"), HC), _chunk_pack(f32("ao_w"), HC)], axis=1).astype(BF)
    wb_pk = np.concatenate([
        _chunk_pack(f32("int_w"), HC), _chunk_pack(f32("out_w"), IC)], axis=1).astype(BF)

    smallpk = np.zeros((128, 100), dtype=np.float32)
    smallpk[:, SP_GB1:SP_GB1 + 6] = f32("gc1_b").reshape(6, 128).T
    smallpk[:, SP_GB2:SP_GB2 + 6] = f32("gc2_b").reshape(6, 128).T
    smallpk[:, SP_PJB:SP_PJB + 6] = f32("proj_b").reshape(6, 128).T
    smallpk[:DH, SP_QB:SP_QB + NH] = f32("q_b").reshape(NH, DH).T
    smallpk[:DH, SP_KB:SP_KB + NH] = f32("k_b").reshape(NH, DH).T
    smallpk[:, SP_INTB:SP_INTB + IC] = f32("int_b").reshape(IC, 128).T
    smallpk[:, SP_DW:SP_DW + HC * NL] = f32("dec_w").reshape(HC, 128, NL).transpose(1, 0, 2).reshape(128, HC * NL)

    biasbc = np.concatenate([
        f32("ao_b"), f32("v_b"), f32("out_b"),
        f32("ln1_g"), f32("ln1_b"), f32("ln2_g"), f32("ln2_b"),
        f32("dec_b")]).astype(np.float32)
    assert biasbc.shape[0] == NBB

    in_maps = []
    boff = (np.arange(BB) * S).astype(np.int32)[:, None, None]
    for i in range(NCORES):
        sl = slice(BB * i, BB * i + BB)
        cnm_i = cn[sl].astype(np.float32).reshape(NCL)
        # attention mask: -1e4 for masked clauses within a batch block,
        # -3e4 for cross-batch entries (forces exp() to exactly 0).
        am = np.full((NCL, NCL), -30000.0, dtype=np.float32)
        for b in range(BB):
            blk = (1.0 - cnm_i[b * M : (b + 1) * M]) * -10000.0
            am[b * M : (b + 1) * M, b * M : (b + 1) * M] = blk[None, :]
        amask8 = np.repeat(am[:, None, :], NH, axis=1).reshape(NCL, NH * NCL)
        percl = np.concatenate([
            amask8, tl[sl].reshape(NCL, NL), cnm_i[:, None]], axis=1)

        gidx = (wr[sl] + boff).reshape(NROW).astype(np.int32)
        wrm_f = wm[sl].astype(np.float32).reshape(NROW)

        # pre-gathered, masked clause tokens, already transposed to the
        # XmT layout: [128 h-part, group, h-chunk, 512 rows]
        xtg_full = enc[sl].reshape(BB * S, H)[gidx] * wrm_f[:, None]
        xtg_h = xtg_full.reshape(4, 512, HC, 128).transpose(3, 0, 2, 1).reshape(
            128, RT * H).astype(BF)

        # per-row avg-pool weights wrm/len, one column per clause-within-tile
        lens = np.maximum(wrm_f.reshape(NCL, LC).sum(1), 1.0)
        aux_h = np.concatenate([
            np.tile((1.0 / lens)[None, :], (128, 1)),
            f32("dec_w").reshape(HC, 128, NL).transpose(1, 0, 2).reshape(128, HC * NL),
        ], axis=1).astype(BF)

        adjc = adj[sl].reshape(NCL, LC, LC)
        adjT = np.zeros((RT, 128, 128), dtype=np.float32)
        for q in range(NCL):
            r, ii = q // 4, q % 4
            adjT[r, 32 * ii : 32 * ii + 32, 32 * ii : 32 * ii + 32] = adjc[q].T
        adjT = adjT.transpose(1, 0, 2).reshape(128, RT * 128).astype(BF)

        d = dict(
            xtg=np.ascontiguousarray(xtg_h),
            wrmb=wrm_f.astype(BF),
            aux=np.ascontiguousarray(aux_h),
            adjt=np.ascontiguousarray(adjT),
            wg=wg_pk, wa=wa_pk, wb=wb_pk,
            smallpk=smallpk, biasbc=biasbc,
            percl=np.ascontiguousarray(percl),
        )
        in_maps.append(d)
    return in_maps


def run_spmd(inputs, trace=False):
    nc = _get_program()
    in_maps = shard_inputs(inputs)
    kw = {}
    if trace:
        import types
        from trn_agent_boot.trn_boot import _ntff_profile_via_ctypes
        mod = types.ModuleType("antenv.axon_hooks")
        hook = _ntff_profile_via_ctypes("/opt/axon/libaxon_pjrt.so")
        mod.get_axon_ntff_profile_hook = lambda: hook
        mod.set_axon_ntff_profile_hook = lambda h: None
        sys.modules["antenv.axon_hooks"] = mod
        bass_utils.upload_artifacts = lambda tmpdir: "local://" + tmpdir
        kw["trace"] = True
    res = bass_utils.run_bass_kernel_spmd(nc, in_maps, core_ids=list(range(NCORES)), **kw)
    return res


def kernel(**inputs):
    res = run_spmd(inputs)
    num = 0.0
    den = 0.0
    for i in range(NCORES):
        o = res.results[i]["out"]
        num += float(o[0])
        den += float(o[1])
    loss = (num / NL) / den
    return np.asarray(loss, dtype=np.float32)
